# revision 4
# baseline (speedup 1.0000x reference)
"""Trainium2 Bass kernel for the SNN (snntorch Leaky, subtract-reset) forward pass.

Computation (reference):
    cur1 = x @ W1.T + b1                      # [B, 100], static across steps
    25 steps of:  reset1 = H(mem1 - 1); mem1 = 0.95*mem1 + cur1 - reset1
                  spk1 = H(mem1 - 1);   cur2 = spk1 @ W2.T + b2
                  reset2 = H(mem2 - 1); mem2 = 0.95*mem2 + cur2 - reset2
    returns mem2 per step: [25, B, 2]

Device algorithm (per core, batch shard of 8192, hidden dim on partitions):
  Change of variables kills the per-step cur1 re-add:
    A = cur1/(1-beta), z = mem1 - A  =>  z' = beta*z - spk,  spk = H(z - theta),
    theta = 1 - A (per-element constant).
  Scaling Z~_t = beta^-t * z_t makes the update a pure subtract that the PE
  applies as an accumulating matmul with stationary -beta^-t * I:
    Z~_t = Z~_{t-1} - beta^-t * spk_{t-1},   spk_t = H(Z~_t - beta^-t * theta).
  Per step: ACT rescales theta (*1/beta), DVE does the compare (PSUM vs SBUF),
  PE applies the spike-subtract and the fc2 matmul (W2 stacked with b2 via a
  constant-ones row in the spike tile). cur2 for all 25 steps accumulates in a
  [50, cols] PSUM tile (rows 2t:2t+2), which is then PE-transposed to
  batch-major [128, 50] tiles for the cheap mem2 recurrence, and DMA'd out.
"""

import numpy as np

BETA = 0.95
T = 25
NI, NH, NO = 9, 100, 2
B = 65536
NCORES = 8
SH = B // NCORES          # batch shard per core
f32 = np.float32

_CACHE = {}
_LAST_RESULT = None       # test.py pokes at these for its timing harness
_LAST_IN_MAPS = None
_LAST_NC = None


def _build_nop_nc():
    """Minimal kernel (one tiny DMA round-trip) for dispatch-overhead baseline."""
    import concourse.bass as bass
    import concourse.tile as tile
    from concourse import bacc, mybir
    f32d = mybir.dt.float32
    nc = bacc.Bacc("TRN2", target_bir_lowering=False, debug=False,
                   num_devices=NCORES)
    i_d = nc.dram_tensor("nin", [1, 128], f32d, kind="ExternalInput").ap()
    o_d = nc.dram_tensor("nout", [1, 128], f32d, kind="ExternalOutput").ap()
    with tile.TileContext(nc) as tc:
        with tc.tile_pool(name="sb", bufs=1) as sb:
            tl = sb.tile([1, 128], f32d)
            nc.sync.dma_start(tl[:], i_d[:])
            nc.sync.dma_start(o_d[:], tl[:])
    nc.compile()
    return nc


def _build_nc(sh, rc):
    """Build + compile the Bass program for shard size `sh`, round width `rc`."""
    import concourse.bass as bass
    import concourse.tile as tile
    from concourse import bacc, mybir

    f32d = mybir.dt.float32
    Copy = mybir.ActivationFunctionType.Copy
    Alu = mybir.AluOpType
    nr = sh // rc             # rounds
    nj = sh // 128            # 128-column groups (transpose tiles)
    nblk = rc // 512          # 512-col matmul blocks per round

    nc = bacc.Bacc("TRN2", target_bir_lowering=False, debug=False,
                   num_devices=NCORES)

    xt_d = nc.dram_tensor("xt", [NI + 1, sh], f32d, kind="ExternalInput").ap()
    iw_d = nc.dram_tensor("iw", [NI + 1, NH], f32d, kind="ExternalInput").ap()
    # W2/b2 scattered into per-step [101, 50] stationaries (rows 2t-2:2t
    # nonzero) so every cur2 matmul writes PSUM at partition base 0 --
    # walrus rejects matmul outputs at non-32-aligned partition offsets.
    w2_d = nc.dram_tensor("w2", [NH + 1, T * 2 * T], f32d,
                          kind="ExternalInput").ap()
    up_d = nc.dram_tensor("up", [NH, (T - 1) * NH], f32d,
                          kind="ExternalInput").ap()
    id_d = nc.dram_tensor("ident", [128, 128], f32d, kind="ExternalInput").ap()
    out_d = nc.dram_tensor("out", [T, sh, NO], f32d, kind="ExternalOutput").ap()

    with tile.TileContext(nc) as tc:
        with tc.tile_pool(name="const", bufs=1) as cp, \
             tc.tile_pool(name="work", bufs=2) as wp, \
             tc.tile_pool(name="big", bufs=1) as bp:

            xt = cp.tile([NI + 1, sh], f32d)
            iw = cp.tile([NI + 1, NH], f32d)
            w2 = cp.tile([NH + 1, T * 2 * T], f32d)
            up = cp.tile([NH, (T - 1) * NH], f32d)
            ident = cp.tile([128, 128], f32d)
            nc.sync.dma_start(xt[:], xt_d[:])
            nc.sync.dma_start(iw[:], iw_d[:])
            nc.sync.dma_start(w2[:], w2_d[:])
            nc.sync.dma_start(up[:], up_d[:])
            nc.sync.dma_start(ident[:], id_d[:])

            cur2 = bp.tile([2 * T, sh], f32d)      # cur2_t for all steps/cols

            ps_rounds = tc.tile_pool(name="psA", bufs=1,
                                     space=bass.MemorySpace.PSUM)
            ps = ps_rounds.__enter__()
            for r in range(nr):
                cs = slice(r * rc, (r + 1) * rc)
                zt = ps.tile([NH, rc], f32d, tag="zt")
                c2 = ps.tile([2 * T, rc], f32d, tag="c2")
                z0 = wp.tile([NH, rc], f32d, tag="z0")
                spks = [wp.tile([NH + 1, rc], f32d, tag=f"spk{i}",
                                 name=f"spk{i}_{r}") for i in range(2)]

                # ones row (partition 100) for the b2 / bias trick; rows
                # 96..99 get overwritten by every compare, which is fine.
                for s_ in spks:
                    nc.vector.memset(s_[96:NH + 1, :], 1.0)

                # Z~_0 = -A = (-W1/(1-beta)) @ x + (-b1/(1-beta))
                for k in range(nblk):
                    bs = slice(k * 512, (k + 1) * 512)
                    nc.tensor.matmul(zt[:, bs], iw[:],
                                     xt[:, r * rc + k * 512:
                                        r * rc + (k + 1) * 512],
                                     start=True, stop=True)
                # keep Z~_0 in SBUF so theta_t can be rebuilt exactly each
                # step (avoids 25x accumulated rescale rounding)
                nc.scalar.activation(z0[:], zt[:], Copy, bias=0.0, scale=1.0)

                half = rc // 2
                sl_a, sl_b = slice(0, half), slice(half, rc)
                for t in range(1, T + 1):
                    # theta_t = beta^-t * (1 + Z~_0); slice A on ScalarE,
                    # slice B on GPSIMD so neither engine is the bottleneck
                    bt = float(f32(np.float64(BETA) ** -t))
                    tha = wp.tile([NH, half], f32d, tag="tha", name=f"tha_{r}_{t}")
                    thb = wp.tile([NH, half], f32d, tag="thb", name=f"thb_{r}_{t}")
                    nc.scalar.activation(tha[:], z0[:, sl_a], Copy, bias=bt,
                                         scale=bt)
                    nc.gpsimd.tensor_scalar(thb[:], z0[:, sl_b], bt, bt,
                                            Alu.mult, Alu.add)
                    spk = spks[t % 2]
                    spkp = spks[(t - 1) % 2]
                    if t >= 2:
                        us = slice((t - 2) * NH, (t - 1) * NH)
                        for k in range(nblk):
                            bs = slice(k * 512, (k + 1) * 512)
                            nc.tensor.matmul(zt[:, bs], up[:, us],
                                             spkp[0:NH, bs], start=False,
                                             stop=True, skip_group_check=True)
                    # spk_t = (Z~_t > theta_t), halves pipeline vs PE
                    nc.vector.tensor_tensor(spk[0:NH, sl_a], zt[:, sl_a],
                                            tha[:], Alu.is_gt)
                    nc.vector.tensor_tensor(spk[0:NH, sl_b], zt[:, sl_b],
                                            thb[:], Alu.is_gt)
                    # cur2_t = W2 @ spk_t + b2  (ones row supplies b2); the
                    # step-t stationary is zero outside rows 2t-2:2t, so the
                    # accumulating matmul only touches its own row pair.
                    ws = slice((t - 1) * 2 * T, t * 2 * T)
                    for k in range(nblk):
                        bs = slice(k * 512, (k + 1) * 512)
                        nc.tensor.matmul(c2[:, bs], w2[:, ws], spk[:, bs],
                                         start=(t == 1), stop=True,
                                         skip_group_check=True)

                nc.scalar.activation(cur2[:, cs], c2[:], Copy, bias=0.0,
                                     scale=1.0)

            ps_rounds.__exit__(None, None, None)

            # ---- mem2 phase: transpose to batch-major, run the recurrence --
            ps_m2 = tc.tile_pool(name="psB", bufs=1,
                                 space=bass.MemorySpace.PSUM)
            ps2 = ps_m2.__enter__()
            m2 = ps2.tile([128, nj, 64], f32d, tag="m2")
            for j in range(nj):
                nc.tensor.transpose(m2[:, j, 0:2 * T],
                                    cur2[:, j * 128:(j + 1) * 128],
                                    ident[0:2 * T, 0:2 * T])

            # mem2_1 = cur2_1 already in place (rows 0:2).
            for t in range(2, T + 1):
                vp = m2[:, :, 2 * t - 4:2 * t - 2]   # mem2_{t-1}
                vt = m2[:, :, 2 * t - 2:2 * t]       # cur2_t -> mem2_t
                r2 = wp.tile([128, nj, 2], f32d, tag="r2")
                u = wp.tile([128, nj, 2], f32d, tag="u")
                nc.vector.tensor_single_scalar(r2[:], vp, 1.0, Alu.is_gt)
                # u = beta * mem2_{t-1} - reset2_t
                nc.vector.scalar_tensor_tensor(u[:], vp, float(BETA), r2[:],
                                               Alu.mult, Alu.subtract)
                nc.vector.tensor_tensor(vt, vt, u[:], Alu.add)

            # ---- reorder (t, j, o) and DMA out ---------------------------
            osb = bp.tile([128, T, nj, NO], f32d)
            src = m2[:, :, 0:2 * T].rearrange("p j (t o) -> p t j o", o=2)
            nc.vector.tensor_copy(osb[:], src)
            dst = out_d.rearrange("t (p j) o -> p t j o", p=128)
            nc.sync.dma_start(dst, osb[:])
            ps_m2.__exit__(None, None, None)

    nc.compile()
    return nc


def _get_nc(sh, rc):
    key = (sh, rc)
    if key not in _CACHE:
        _CACHE[key] = _build_nc(sh, rc)
    return _CACHE[key]


def _host_consts(W1, b1, W2, b2):
    inv = 1.0 / (1.0 - np.float64(BETA))
    iw = np.zeros((NI + 1, NH), f32)
    iw[0:NI] = (-W1.astype(np.float64).T * inv).astype(f32)
    iw[NI] = (-b1.astype(np.float64) * inv).astype(f32)
    w2e = np.zeros((NH + 1, T * 2 * T), f32)
    for t in range(1, T + 1):
        w2e[0:NH, (t - 1) * 2 * T + 2 * t - 2:(t - 1) * 2 * T + 2 * t] = W2.T
        w2e[NH, (t - 1) * 2 * T + 2 * t - 2:(t - 1) * 2 * T + 2 * t] = b2
    up = np.zeros((NH, (T - 1) * NH), f32)
    idx = np.arange(NH)
    for t in range(2, T + 1):
        coef = f32(-(np.float64(BETA) ** -t))
        up[idx, (t - 2) * NH + idx] = coef
    ident = np.eye(128, dtype=f32)
    return iw, w2e, up, ident


def kernel(x, W1, b1, W2, b2):
    global _LAST_RESULT, _LAST_IN_MAPS, _LAST_NC
    from concourse.bass_utils import run_bass_kernel_spmd

    x = np.ascontiguousarray(x, f32)
    W1 = np.asarray(W1, f32)
    b1 = np.asarray(b1, f32)
    W2 = np.asarray(W2, f32)
    b2 = np.asarray(b2, f32)

    sh, rc = SH, 2048
    nc = _get_nc(sh, rc)
    iw, w2e, up, ident = _host_consts(W1, b1, W2, b2)

    # column c of the device layout holds batch element perm[c]; chosen so the
    # output DMA writes 512B-contiguous DRAM chunks per partition.
    cols = np.arange(sh)
    perm = (cols % 128) * (sh // 128) + cols // 128

    in_maps = []
    for i in range(NCORES):
        xs = x[i * sh:(i + 1) * sh]
        xt = np.ones((NI + 1, sh), f32)
        xt[0:NI] = xs[perm].T
        in_maps.append({"xt": xt, "iw": iw, "w2": w2e, "up": up,
                        "ident": ident})

    _LAST_IN_MAPS = in_maps
    _LAST_NC = nc
    res = run_bass_kernel_spmd(nc, in_maps, list(range(NCORES)))
    _LAST_RESULT = res
    return np.concatenate([res.results[i]["out"] for i in range(NCORES)],
                          axis=1)



# revision 20
# speedup vs baseline: 2.9171x; 2.9171x over previous
"""Trainium2 Bass kernel for the SNN (snntorch Leaky, subtract-reset) forward.

Single fp32r matmul pass per step folds everything into one PE stream:
    P_t(h,b) accumulates  sum_s d_s*A(h,b) - sum_s c_s*spk_{s-1}
  where A = (W1 x + b1)/(1-beta), d_s = beta^-s (1-beta), c_s = beta^-s.
  Spike condition: mem_t > 1  <=>  P_t > tau_t = beta^-t  (scalar!).
Moving rows [128] = [spikes(100); x_hi(9); x_lo(9); x_hi strong8(8); 1; 1].
The x hi/lo/cross split delivers the theta decrement to ~22 bits despite
fp32r's ~11-bit ingest rounding. The spike subtract is exact via a two-
factor trick: the DVE compare writes spk in {0, K_t} (tensor_scalar
is_gt/mult) and the diag coefficient D_{t+1} is chosen so D_{t+1}*K_t ~
beta^-(t+1) to ~2^-22 (both factors 11-bit representable).
cur2 accumulates in PSUM rows 100:124 (6 pairs x 2 outs x hi/lo W2 split);
cumulative pair readouts after passes 7/13/19/25/26 are differenced in the
transposed post-phase, which also runs the cheap mem2 recurrence.
"""

import numpy as np

BETA = 0.95
T = 25
NPASS = 26
NI, NH, NO = 9, 100, 2
B = 65536
NCORES = 8
SH = B // NCORES          # 8192
RC = 2048
NR = SH // RC             # 4 rounds
NBLK = RC // 512
NJ = SH // 128            # 64 transpose blocks
f32 = np.float32
f64 = np.float64

_CACHE = {}
_LAST_RESULT = None
_LAST_IN_MAPS = None
_LAST_NC = None

RO_PASS = {7: 0, 13: 1, 19: 2, 25: 3, 26: 4}   # pass -> readout index


def _rnd11(v):
    """Round fp32 mantissa to 11 bits (half-up). Survives HW fp32r ingest."""
    v = np.asarray(v, f32)
    u = v.view(np.uint32).astype(np.uint64)
    u = (u + (np.uint64(1) << np.uint64(11))) >> np.uint64(12) << np.uint64(12)
    return (u & np.uint64(0xFFFFFFFF)).astype(np.uint32).view(f32)


def _find_DK(c):
    """11-bit pair (D, K) with D*K ~ c to ~2^-22."""
    best = (None, None, 1e9)
    for k in range(2048):
        K = 1.0 + k / 2048.0
        D = f64(_rnd11(f32(c / K)))
        e = abs(D * K - c)
        if e < best[2]:
            best = (D, K, e)
    return best[0], best[1]


def _host_consts(W1, b1, W2, b2):
    inv = 1.0 / (1.0 - f64(BETA))
    W1 = W1.astype(f64)
    b1 = b1.astype(f64)
    W2 = W2.astype(f64)
    b2 = b2.astype(f64)
    strong8 = np.argsort(-np.linalg.norm(W1, axis=0))[:8]

    # pair factorization: D_t * K_{t-1} ~ beta^-t  (t = 2..25)
    Kf = np.ones(T + 1, f64)          # Kf[t] scales spk_t (t = 1..25)
    Dg = np.zeros(NPASS + 1, f64)     # Dg[t] diag at pass t
    for t in range(2, T + 1):
        D, K = _find_DK(f64(BETA) ** -t)
        Dg[t] = D
        Kf[t - 1] = K
    tau = np.array([0.0] + [f32(f64(BETA) ** -t) for t in range(1, T + 1)],
                   f32)

    S = np.zeros((NPASS, 128, 124), f32)
    for t in range(1, NPASS + 1):
        s = S[t - 1]
        if 2 <= t:
            sp = t - 1                    # spike step consumed by this pass
            K = Kf[sp]
            if t <= T:
                np.fill_diagonal(s[0:NH, 0:NH], f32(-Dg[t]))
            p = (t - 2) % 6
            w2h = _rnd11((W2 / K).astype(f32))
            w2l = _rnd11(((W2 - w2h.astype(f64) * K) / K).astype(f32))
            for o in range(NO):
                s[0:NH, 100 + 2 * p + o] = w2h[o]
                s[0:NH, 112 + 2 * p + o] = w2l[o]
                s[126, 100 + 2 * p + o] = _rnd11(f32(b2[o]))
        if t <= T:
            d_t = f64(BETA) ** -t * (1.0 - f64(BETA))
            cf = d_t * W1 * inv                      # [NH, NI]
            chx = _rnd11(cf.astype(f32))
            clx = _rnd11((cf - chx.astype(f64)).astype(f32))
            ca = d_t * b1 * inv
            cah = _rnd11(ca.astype(f32))
            cal = _rnd11((ca - cah.astype(f64)).astype(f32))
            s[100:109, 0:NH] = chx.T
            s[109:118, 0:NH] = chx.T
            s[118:126, 0:NH] = clx.T[strong8]
            s[126, 0:NH] += cah
            s[127, 0:NH] = cal
    sm = S.transpose(1, 0, 2).reshape(128, NPASS * 124)
    return sm, tau, Kf.astype(f32), strong8


def _build_nc(tau, kf):
    import concourse.bass as bass
    import concourse.tile as tile
    from concourse import bacc, mybir

    f32d = mybir.dt.float32
    f32r = mybir.dt.float32r
    Copy = mybir.ActivationFunctionType.Copy
    Alu = mybir.AluOpType

    nc = bacc.Bacc("TRN2", target_bir_lowering=False, debug=False,
                   num_devices=NCORES)

    xt3_d = nc.dram_tensor("xt3", [28, SH], f32r, kind="ExternalInput").ap()
    sm_d = nc.dram_tensor("sm", [128, NPASS * 124], f32r,
                          kind="ExternalInput").ap()
    id_d = nc.dram_tensor("ident", [128, 128], f32d,
                          kind="ExternalInput").ap()
    out_d = nc.dram_tensor("out", [T, SH, NO], f32d,
                           kind="ExternalOutput").ap()

    with tile.TileContext(nc) as tc:
        with tc.tile_pool(name="const", bufs=1) as cp, \
             tc.tile_pool(name="ro", bufs=1) as rp:
            sm = cp.tile([128, NPASS * 124], f32r)
            ident = cp.tile([128, 128], f32d)
            nc.sync.dma_start(sm[:], sm_d[:])
            nc.sync.dma_start(ident[:], id_d[:])
            ro = rp.tile([120, SH], f32d)
            scr = [rp.tile([128, RC], f32d, name=f"scr{i}") for i in range(2)]

            with tc.tile_pool(name="spk", bufs=1) as kp:
                spks = [[kp.tile([128, RC], f32r, name=f"spk_{r}_{i}")
                         for i in range(2)] for r in range(NR)]
                for r in range(NR):
                    cs = slice(r * RC, (r + 1) * RC)
                    for i in range(2):
                        nc.sync.dma_start(spks[r][i][100:128, :],
                                          xt3_d[:, cs])
                    # pass 1 reads the parity-0 tile with zero spike coefs;
                    # garbage would still poison PSUM via 0*NaN
                    nc.vector.memset(spks[r][0][0:NH, :].bitcast(f32d), 0.0)

                for ph in range(2):
                    ps_pool = tc.tile_pool(name=f"ps{ph}", bufs=1,
                                           space=bass.MemorySpace.PSUM)
                    ps = ps_pool.__enter__()
                    rr = (2 * ph, 2 * ph + 1)
                    P = {r: ps.tile([128, RC], f32d, tag=f"P{r}",
                                    name=f"P_{r}") for r in rr}
                    for t in range(1, NPASS + 1):
                        st = sm[:, (t - 1) * 124:t * 124]
                        for r in rr:
                            mv = spks[r][(t - 1) % 2]
                            for b in range(NBLK):
                                bs = slice(b * 512, (b + 1) * 512)
                                nc.tensor.matmul(
                                    P[r][0:124, bs], st, mv[:, bs],
                                    start=(t == 1), stop=True,
                                    skip_group_check=True)
                            if t <= T:
                                nc.vector.tensor_scalar(
                                    spks[r][t % 2][0:NH, :], P[r][0:NH, :],
                                    float(tau[t]), float(kf[t]),
                                    Alu.is_gt, Alu.mult)
                            if t in RO_PASS:
                                k = RO_PASS[t]
                                sc = scr[r % 2]
                                nc.scalar.activation(sc[96:128, :],
                                                     P[r][96:128, :], Copy,
                                                     bias=0.0, scale=1.0)
                                nc.sync.dma_start(
                                    ro[24 * k:24 * k + 24,
                                       r * RC:(r + 1) * RC],
                                    sc[100:124, :])
                    ps_pool.__exit__(None, None, None)

            # ---- post: transpose, hi+lo, diff, mem2 recurrence, output ----
            with tc.tile_pool(name="post", bufs=1) as pp, \
                 tc.tile_pool(name="psT", bufs=1,
                              space=bass.MemorySpace.PSUM) as pt:
                tt = pp.tile([128, NJ, 120], f32d)
                for half in range(2):
                    # 128-col slots: each transpose output stays inside one
                    # 2KB PSUM bank (120-col slots would cross banks)
                    ptile = pt.tile([128, NJ // 2, 128], f32d, tag="tp",
                                    name=f"tp_{half}")
                    for jj in range(NJ // 2):
                        j = half * (NJ // 2) + jj
                        nc.tensor.transpose(
                            ptile[:, jj, 0:120],
                            ro[:, j * 128:(j + 1) * 128],
                            ident[0:120, 0:120])
                    nc.scalar.activation(
                        tt[:, half * (NJ // 2):(half + 1) * (NJ // 2), :],
                        ptile[:, :, 0:120], Copy, bias=0.0, scale=1.0)

                # s = hi + lo  -> [128, NJ, 60] (cols 12k + 2p + o)
                ssb = pp.tile([128, NJ, 60], f32d)
                tv = tt[:].rearrange("p j (k q) -> p (j k) q", q=24)
                sv = ssb[:].rearrange("p j (k q) -> p (j k) q", q=12)
                nc.vector.tensor_tensor(sv, tv[:, :, 0:12], tv[:, :, 12:24],
                                        Alu.add)
                # diff across consecutive readouts -> cur2 for t >= 7
                dsb = pp.tile([128, NJ, 48], f32d)
                nc.vector.tensor_tensor(dsb[:], ssb[:, :, 12:60],
                                        ssb[:, :, 0:48], Alu.subtract)

                osb = pp.tile([128, T, NJ, NO], f32d)
                r2 = pp.tile([128, NJ, NO], f32d)
                u = pp.tile([128, NJ, NO], f32d)
                for t in range(1, T + 1):
                    k, p = (t - 1) // 6, (t - 1) % 6
                    if k == 0:
                        cv = ssb[:, :, 2 * p:2 * p + 2]
                    else:
                        cv = dsb[:, :, 12 * (k - 1) + 2 * p:
                                 12 * (k - 1) + 2 * p + 2]
                    if t == 1:
                        nc.vector.tensor_copy(osb[:, 0, :, :], cv)
                        continue
                    pm = osb[:, t - 2, :, :]
                    nc.vector.tensor_single_scalar(r2[:], pm, 1.0, Alu.is_gt)
                    nc.vector.scalar_tensor_tensor(u[:], pm, float(BETA),
                                                   r2[:], Alu.mult,
                                                   Alu.subtract)
                    nc.vector.tensor_tensor(osb[:, t - 1, :, :], u[:], cv,
                                            Alu.add)
                dst = out_d.rearrange("t (p j) o -> p t j o", p=128)
                nc.sync.dma_start(dst, osb[:])

    nc.compile()
    return nc


def _get_nc(tau, kf):
    key = "v2"
    if key not in _CACHE:
        _CACHE[key] = _build_nc(tau, kf)
    return _CACHE[key]


def kernel(x, W1, b1, W2, b2):
    global _LAST_RESULT, _LAST_IN_MAPS, _LAST_NC
    from concourse.bass_utils import run_bass_kernel_spmd

    x = np.ascontiguousarray(x, f32)
    sm, tau, kf, strong8 = _host_consts(np.asarray(W1, f32),
                                        np.asarray(b1, f32),
                                        np.asarray(W2, f32),
                                        np.asarray(b2, f32))
    nc = _get_nc(tau, kf)

    cols = np.arange(SH)
    perm = (cols % 128) * (SH // 128) + cols // 128
    ident = np.eye(128, dtype=f32)

    in_maps = []
    for i in range(NCORES):
        xs = x[i * SH:(i + 1) * SH][perm]          # [SH, 9]
        x_hi = _rnd11(xs)
        x_lo = _rnd11(xs - x_hi)
        xt3 = np.ones((28, SH), f32)
        xt3[0:9] = x_hi.T
        xt3[9:18] = x_lo.T
        xt3[18:26] = x_hi.T[strong8]
        in_maps.append({"xt3": xt3, "sm": sm, "ident": ident})

    _LAST_IN_MAPS = in_maps
    _LAST_NC = nc
    res = run_bass_kernel_spmd(nc, in_maps, list(range(NCORES)))
    _LAST_RESULT = res
    return np.concatenate([res.results[i]["out"] for i in range(NCORES)],
                          axis=1)


# revision 26
# speedup vs baseline: 3.0178x; 1.0345x over previous
"""Trainium2 Bass kernel for the SNN (snntorch Leaky, subtract-reset) forward.

Single fp32r matmul pass per step folds everything into one PE stream:
    P_t(h,b) accumulates  sum_s d_s*A(h,b) - sum_s c_s*spk_{s-1}
  where A = (W1 x + b1)/(1-beta), d_s = beta^-s (1-beta), c_s = beta^-s.
  Spike condition: mem_t > 1  <=>  P_t > tau_t = beta^-t  (scalar!).
Moving rows [128] = [spikes(100); x_hi(9); x_lo(9); x_hi strong8(8); 1; 1].
The x hi/lo/cross split delivers the theta decrement to ~22 bits despite
fp32r's ~11-bit ingest rounding. The spike subtract is exact via a two-
factor trick: the DVE compare writes spk in {0, K_t} (tensor_scalar
is_gt/mult) and the diag coefficient D_{t+1} is chosen so D_{t+1}*K_t ~
beta^-(t+1) to ~2^-22 (both factors 11-bit representable).
cur2 accumulates in PSUM rows 100:124 (6 pairs x 2 outs x hi/lo W2 split);
cumulative pair readouts after passes 7/13/19/25/26 are differenced in the
transposed post-phase, which also runs the cheap mem2 recurrence.
"""

import numpy as np

BETA = 0.95
T = 25
NPASS = 26
NI, NH, NO = 9, 100, 2
B = 65536
NCORES = 8
SH = B // NCORES          # 8192
RC = 2048
NR = SH // RC             # 4 rounds
NBLK = RC // 512
NJ = SH // 128            # 64 transpose blocks
f32 = np.float32
f64 = np.float64

_CACHE = {}
_LAST_RESULT = None
_LAST_IN_MAPS = None
_LAST_NC = None

RO_PASS = {7: 0, 13: 1, 19: 2, 25: 3, 26: 4}   # pass -> readout index


def _rnd11(v):
    """Round fp32 mantissa to 11 bits (half-up). Survives HW fp32r ingest."""
    v = np.asarray(v, f32)
    u = v.view(np.uint32).astype(np.uint64)
    u = (u + (np.uint64(1) << np.uint64(11))) >> np.uint64(12) << np.uint64(12)
    return (u & np.uint64(0xFFFFFFFF)).astype(np.uint32).view(f32)


def _find_DK(c):
    """11-bit pair (D, K) with D*K ~ c to ~2^-22."""
    best = (None, None, 1e9)
    for k in range(2048):
        K = 1.0 + k / 2048.0
        D = f64(_rnd11(f32(c / K)))
        e = abs(D * K - c)
        if e < best[2]:
            best = (D, K, e)
    return best[0], best[1]


def _host_consts(W1, b1, W2, b2):
    inv = 1.0 / (1.0 - f64(BETA))
    W1 = W1.astype(f64)
    b1 = b1.astype(f64)
    W2 = W2.astype(f64)
    b2 = b2.astype(f64)
    strong8 = np.argsort(-np.linalg.norm(W1, axis=0))[:8]

    # pair factorization: D_t * K_{t-1} ~ beta^-t  (t = 2..25)
    Kf = np.ones(T + 1, f64)          # Kf[t] scales spk_t (t = 1..25)
    Dg = np.zeros(NPASS + 1, f64)     # Dg[t] diag at pass t
    for t in range(2, T + 1):
        D, K = _find_DK(f64(BETA) ** -t)
        Dg[t] = D
        Kf[t - 1] = K
    tau = np.array([0.0] + [f32(f64(BETA) ** -t) for t in range(1, T + 1)],
                   f32)

    # D-variant: DVE columns, spikes in {0, K_t} (exact two-factor subtract).
    # A-variant: ACT columns, spikes in {-1, +1} from Sign (11-bit subtract).
    S = np.zeros((NPASS, 128, 124), f32)
    SA = np.zeros((NPASS, 128, 124), f32)
    for t in range(1, NPASS + 1):
        s = S[t - 1]
        sa = SA[t - 1]
        if 2 <= t:
            sp = t - 1                    # spike step consumed by this pass
            K = Kf[sp]
            if t <= T:
                np.fill_diagonal(s[0:NH, 0:NH], f32(-Dg[t]))
                ch2 = _rnd11(f32(f64(BETA) ** -t / 2.0))
                np.fill_diagonal(sa[0:NH, 0:NH], f32(-ch2))
                sa[126, 0:NH] -= ch2
            p = (t - 2) % 6
            w2h = _rnd11((W2 / K).astype(f32))
            w2l = _rnd11(((W2 - w2h.astype(f64) * K) / K).astype(f32))
            w2h2 = _rnd11((W2 / 2.0).astype(f32))
            w2l2 = _rnd11((W2 / 2.0 - w2h2.astype(f64)).astype(f32))
            for o in range(NO):
                s[0:NH, 100 + 2 * p + o] = w2h[o]
                s[0:NH, 112 + 2 * p + o] = w2l[o]
                s[126, 100 + 2 * p + o] = _rnd11(f32(b2[o]))
                sa[0:NH, 100 + 2 * p + o] = w2h2[o]
                sa[0:NH, 112 + 2 * p + o] = w2l2[o]
                sa[126, 100 + 2 * p + o] = _rnd11(
                    f32(b2[o] + W2[o].sum() / 2.0))
        if t <= T:
            d_t = f64(BETA) ** -t * (1.0 - f64(BETA))
            cf = d_t * W1 * inv                      # [NH, NI]
            chx = _rnd11(cf.astype(f32))
            clx = _rnd11((cf - chx.astype(f64)).astype(f32))
            ca = d_t * b1 * inv
            cah = _rnd11(ca.astype(f32))
            cal = _rnd11((ca - cah.astype(f64)).astype(f32))
            for m in (s, sa):
                m[100:109, 0:NH] = chx.T
                m[109:118, 0:NH] = chx.T
                m[118:126, 0:NH] = clx.T[strong8]
                m[126, 0:NH] += cah
                m[127, 0:NH] = cal
    sm = np.concatenate([S.transpose(1, 0, 2).reshape(128, NPASS * 124),
                         SA.transpose(1, 0, 2).reshape(128, NPASS * 124)],
                        axis=1)
    return sm, tau, Kf.astype(f32), strong8


def _build_nc(tau, kf):
    import concourse.bass as bass
    import concourse.tile as tile
    from concourse import bacc, mybir

    f32d = mybir.dt.float32
    f32r = mybir.dt.float32r
    Copy = mybir.ActivationFunctionType.Copy
    Sign = mybir.ActivationFunctionType.Sign
    Alu = mybir.AluOpType

    nc = bacc.Bacc("TRN2", target_bir_lowering=False, debug=False,
                   num_devices=NCORES)

    xt3_d = nc.dram_tensor("xt3", [28, SH], f32r, kind="ExternalInput").ap()
    sm_d = nc.dram_tensor("sm", [128, 2 * NPASS * 124], f32r,
                          kind="ExternalInput").ap()
    ts_d = nc.dram_tensor("taus", [128, T], f32d, kind="ExternalInput").ap()
    id_d = nc.dram_tensor("ident", [128, 128], f32d,
                          kind="ExternalInput").ap()
    out_d = nc.dram_tensor("out", [T, SH, NO], f32d,
                           kind="ExternalOutput").ap()

    with tile.TileContext(nc) as tc:
        with tc.tile_pool(name="const", bufs=1) as cp, \
             tc.tile_pool(name="ro", bufs=1) as rp:
            sm = cp.tile([128, 2 * NPASS * 124], f32r)
            taus = cp.tile([128, T], f32d)
            ident = cp.tile([128, 128], f32d)
            nc.sync.dma_start(sm[:], sm_d[:])
            nc.sync.dma_start(taus[:], ts_d[:])
            nc.sync.dma_start(ident[:], id_d[:])
            ro = rp.tile([120, SH], f32d)
            scr = [rp.tile([128, RC], f32d, name=f"scr{i}") for i in range(2)]

            with tc.tile_pool(name="spk", bufs=1) as kp:
                spks = [[kp.tile([128, RC], f32r, name=f"spk_{r}_{i}")
                         for i in range(2)] for r in range(NR)]
                for r in range(NR):
                    cs = slice(r * RC, (r + 1) * RC)
                    for i in range(2):
                        nc.sync.dma_start(spks[r][i][100:128, :],
                                          xt3_d[:, cs])
                    # pass 1 reads the parity-0 tile with zero spike coefs;
                    # garbage would still poison PSUM via 0*NaN
                    nc.vector.memset(spks[r][0][0:NH, :].bitcast(f32d), 0.0)

                for ph in range(2):
                    ps_pool = tc.tile_pool(name=f"ps{ph}", bufs=1,
                                           space=bass.MemorySpace.PSUM)
                    ps = ps_pool.__enter__()
                    rr = (2 * ph, 2 * ph + 1)
                    P = {r: ps.tile([128, RC], f32d, tag=f"P{r}",
                                    name=f"P_{r}") for r in rr}
                    for t in range(1, NPASS + 1):
                        st = sm[:, (t - 1) * 124:t * 124]
                        sta = sm[:, (NPASS + t - 1) * 124:(NPASS + t) * 124]
                        for r in rr:
                            mv = spks[r][(t - 1) % 2]
                            for b in range(NBLK):
                                bs = slice(b * 512, (b + 1) * 512)
                                nc.tensor.matmul(
                                    P[r][0:124, bs], sta if b == 0 else st,
                                    mv[:, bs], start=(t == 1), stop=True,
                                    skip_group_check=True)
                            if t <= T:
                                # block 0 -> ACT Sign (+-1); rest -> DVE {0,K}
                                nc.scalar.activation(
                                    spks[r][t % 2][0:NH, 0:512],
                                    P[r][0:NH, 0:512], Sign,
                                    bias=taus[0:NH, t - 1:t], scale=1.0)
                                nc.vector.tensor_scalar(
                                    spks[r][t % 2][0:NH, 512:RC],
                                    P[r][0:NH, 512:RC],
                                    float(tau[t]), float(kf[t]),
                                    Alu.is_gt, Alu.mult)
                            if t in RO_PASS:
                                k = RO_PASS[t]
                                sc = scr[r % 2]
                                nc.scalar.activation(sc[96:128, :],
                                                     P[r][96:128, :], Copy,
                                                     bias=0.0, scale=1.0)
                                nc.sync.dma_start(
                                    ro[24 * k:24 * k + 24,
                                       r * RC:(r + 1) * RC],
                                    sc[100:124, :])
                    ps_pool.__exit__(None, None, None)

            # ---- post: transpose, hi+lo, diff, mem2 recurrence, output ----
            with tc.tile_pool(name="post", bufs=1) as pp, \
                 tc.tile_pool(name="psT", bufs=1,
                              space=bass.MemorySpace.PSUM) as pt:
                tt = pp.tile([128, NJ, 120], f32d)
                for half in range(2):
                    # 128-col slots: each transpose output stays inside one
                    # 2KB PSUM bank (120-col slots would cross banks)
                    ptile = pt.tile([128, NJ // 2, 128], f32d, tag="tp",
                                    name=f"tp_{half}")
                    for jj in range(NJ // 2):
                        j = half * (NJ // 2) + jj
                        nc.tensor.transpose(
                            ptile[:, jj, 0:120],
                            ro[:, j * 128:(j + 1) * 128],
                            ident[0:120, 0:120])
                    nc.scalar.activation(
                        tt[:, half * (NJ // 2):(half + 1) * (NJ // 2), :],
                        ptile[:, :, 0:120], Copy, bias=0.0, scale=1.0)

                # s = hi + lo  -> [128, NJ, 60] (cols 12k + 2p + o)
                ssb = pp.tile([128, NJ, 60], f32d)
                tv = tt[:].rearrange("p j (k q) -> p (j k) q", q=24)
                sv = ssb[:].rearrange("p j (k q) -> p (j k) q", q=12)
                nc.vector.tensor_tensor(sv, tv[:, :, 0:12], tv[:, :, 12:24],
                                        Alu.add)
                # diff across consecutive readouts -> cur2 for t >= 7
                dsb = pp.tile([128, NJ, 48], f32d)
                nc.vector.tensor_tensor(dsb[:], ssb[:, :, 12:60],
                                        ssb[:, :, 0:48], Alu.subtract)

                osb = pp.tile([128, T, NJ, NO], f32d)
                r2 = pp.tile([128, NJ, NO], f32d)
                u = pp.tile([128, NJ, NO], f32d)
                for t in range(1, T + 1):
                    k, p = (t - 1) // 6, (t - 1) % 6
                    if k == 0:
                        cv = ssb[:, :, 2 * p:2 * p + 2]
                    else:
                        cv = dsb[:, :, 12 * (k - 1) + 2 * p:
                                 12 * (k - 1) + 2 * p + 2]
                    if t == 1:
                        nc.vector.tensor_copy(osb[:, 0, :, :], cv)
                        continue
                    pm = osb[:, t - 2, :, :]
                    nc.vector.tensor_single_scalar(r2[:], pm, 1.0, Alu.is_gt)
                    nc.vector.scalar_tensor_tensor(u[:], pm, float(BETA),
                                                   r2[:], Alu.mult,
                                                   Alu.subtract)
                    nc.vector.tensor_tensor(osb[:, t - 1, :, :], u[:], cv,
                                            Alu.add)
                dst = out_d.rearrange("t (p j) o -> p t j o", p=128)
                nc.sync.dma_start(dst, osb[:])

    nc.compile()
    return nc


def _get_nc(tau, kf):
    key = "v2"
    if key not in _CACHE:
        _CACHE[key] = _build_nc(tau, kf)
    return _CACHE[key]


def kernel(x, W1, b1, W2, b2):
    global _LAST_RESULT, _LAST_IN_MAPS, _LAST_NC
    from concourse.bass_utils import run_bass_kernel_spmd

    x = np.ascontiguousarray(x, f32)
    sm, tau, kf, strong8 = _host_consts(np.asarray(W1, f32),
                                        np.asarray(b1, f32),
                                        np.asarray(W2, f32),
                                        np.asarray(b2, f32))
    nc = _get_nc(tau, kf)

    cols = np.arange(SH)
    perm = (cols % 128) * (SH // 128) + cols // 128
    ident = np.eye(128, dtype=f32)

    in_maps = []
    for i in range(NCORES):
        xs = x[i * SH:(i + 1) * SH][perm]          # [SH, 9]
        x_hi = _rnd11(xs)
        x_lo = _rnd11(xs - x_hi)
        xt3 = np.ones((28, SH), f32)
        xt3[0:9] = x_hi.T
        xt3[9:18] = x_lo.T
        xt3[18:26] = x_hi.T[strong8]
        taus = np.tile(-tau[1:T + 1], (128, 1)).astype(f32)
        in_maps.append({"xt3": xt3, "sm": sm, "taus": taus, "ident": ident})

    _LAST_IN_MAPS = in_maps
    _LAST_NC = nc
    res = run_bass_kernel_spmd(nc, in_maps, list(range(NCORES)))
    _LAST_RESULT = res
    return np.concatenate([res.results[i]["out"] for i in range(NCORES)],
                          axis=1)
